# revision 2
# baseline (speedup 1.0000x reference)
"""Dequantized mixed-sign int8 GEMM on 8 trn2 NeuronCores.

out = ((x - X_ZP) * X_SCALE) @ ((y - Y_ZP) * Y_SCALE)   [4096 x 4096 x 4096]

Strategy: the shifted operands (x+66, y-160) are integers with magnitude
<= 256, exactly representable in bf16 -> run the GEMM as bf16 x bf16 with
fp32 PSUM accumulation (bf16 tensor-engine peak), folding the zero-point
shifts into the on-chip fp32->bf16 conversion and the scale product into
the PSUM->SBUF copy.

Sharding: 4-way over M x 2-way over N (core (mi, nj), mi in 0..3,
nj in 0..1).  Each core gets x[mi].T ([K, 1024] fp32, transposed on host
so K lands on partitions for the stationary operand) and y[:, nj]
([K, 2048] fp32), and produces a [1024, 2048] fp32 output block.
x.T is converted once into SBUF-resident bf16 tiles; y streams through
in 512-wide blocks, double-buffered.
"""

import sys

if "/opt/trn_rl_repo" not in sys.path:
    sys.path.insert(0, "/opt/trn_rl_repo")

import numpy as np

X_SCALE, X_ZP = 0.03, -66.0
Y_SCALE, Y_ZP = 0.025, 160.0
OUT_SCALE = float(np.float32(X_SCALE) * np.float32(Y_SCALE))

M = K = N = 4096
MI, NJ = 4, 2  # core grid: M split x N split
M_SH, N_SH = M // MI, N // NJ  # 1024, 2048 per core
N_CORES = MI * NJ
NBW = 512  # n-block width (one PSUM bank of fp32)


def build(m_sh=M_SH, n_sh=N_SH, k=K, nbw=NBW):
    """Build + compile the per-core Bass program (SPMD: same NEFF on all cores)."""
    from concourse import bacc, mybir, tile

    f32, bf16 = mybir.dt.float32, mybir.dt.bfloat16
    kp = k // 128  # K tiles of 128
    mo_n = m_sh // 128  # M tiles of 128
    nb_n = n_sh // nbw  # N blocks

    nc = bacc.Bacc("TRN2", target_bir_lowering=False, debug=False)
    xt_d = nc.dram_tensor("xt", (k, m_sh), f32, kind="ExternalInput")
    y_d = nc.dram_tensor("y", (k, n_sh), f32, kind="ExternalInput")
    o_d = nc.dram_tensor("o", (m_sh, n_sh), f32, kind="ExternalOutput")

    with tile.TileContext(nc) as tc:
        with (
            tc.tile_pool(name="xstage", bufs=3) as xstage,
            tc.tile_pool(name="ystage", bufs=4) as ystage,
            tc.tile_pool(name="xbf", bufs=1) as xbfp,
            tc.tile_pool(name="ybf", bufs=2) as ybfp,
            tc.tile_pool(name="opool", bufs=4) as opool,
            tc.tile_pool(name="psum", bufs=4, space="PSUM") as psum,
        ):
            # x.T: load fp32, shift by -X_ZP, cast to bf16; resident for the
            # whole kernel ([128, m_sh] per K tile).
            xbf = []
            for ko in range(kp):
                xs = xstage.tile([128, m_sh], f32, tag="xs")
                nc.sync.dma_start(xs[:], xt_d.ap()[128 * ko : 128 * (ko + 1), :])
                xb = xbfp.tile([128, m_sh], bf16, tag=f"x{ko}")
                nc.vector.tensor_scalar_add(xb[:], xs[:], -X_ZP)
                xbf.append(xb)

            for nb in range(nb_n):
                # y block: load fp32, shift by -Y_ZP, cast to bf16
                ybs = []
                for ko in range(kp):
                    ys = ystage.tile([128, nbw], f32, tag="ys")
                    nc.sync.dma_start(
                        ys[:],
                        y_d.ap()[128 * ko : 128 * (ko + 1), nb * nbw : (nb + 1) * nbw],
                    )
                    yb = ybfp.tile([128, nbw], bf16, tag=f"y{ko}")
                    nc.vector.tensor_scalar_add(yb[:], ys[:], -Y_ZP)
                    ybs.append(yb)

                for mo in range(mo_n):
                    ps = psum.tile([128, nbw], f32, tag="ps")
                    for ko in range(kp):
                        nc.tensor.matmul(
                            ps[:],
                            xbf[ko][:, 128 * mo : 128 * (mo + 1)],
                            ybs[ko][:],
                            start=(ko == 0),
                            stop=(ko == kp - 1),
                        )
                    ot = opool.tile([128, nbw], f32, tag="ot")
                    nc.scalar.activation(
                        ot[:], ps[:], mybir.ActivationFunctionType.Copy,
                        scale=OUT_SCALE,
                    )
                    nc.sync.dma_start(
                        o_d.ap()[128 * mo : 128 * (mo + 1), nb * nbw : (nb + 1) * nbw],
                        ot[:],
                    )

    nc.compile()
    return nc


_nc_cache = None


def _get_nc():
    global _nc_cache
    if _nc_cache is None:
        _nc_cache = build()
    return _nc_cache


def make_in_maps(x: np.ndarray, y: np.ndarray) -> list[dict]:
    x = np.ascontiguousarray(x, dtype=np.float32)
    y = np.ascontiguousarray(y, dtype=np.float32)
    xt_shards = [
        np.ascontiguousarray(x[mi * M_SH : (mi + 1) * M_SH].T) for mi in range(MI)
    ]
    y_shards = [
        np.ascontiguousarray(y[:, nj * N_SH : (nj + 1) * N_SH]) for nj in range(NJ)
    ]
    return [{"xt": xt_shards[i // NJ], "y": y_shards[i % NJ]} for i in range(N_CORES)]


def kernel(x: np.ndarray, y: np.ndarray) -> np.ndarray:
    from concourse import bass_utils

    nc = _get_nc()
    in_maps = make_in_maps(x, y)
    res = bass_utils.run_bass_kernel_spmd(nc, in_maps, core_ids=list(range(N_CORES)))

    out = np.empty((M, N), dtype=np.float32)
    for i in range(N_CORES):
        mi, nj = i // NJ, i % NJ
        out[mi * M_SH : (mi + 1) * M_SH, nj * N_SH : (nj + 1) * N_SH] = res.results[i][
            "o"
        ]
    return out
